# revision 40
# baseline (speedup 1.0000x reference)
"""Trainium2 Bass kernel for the req_to_token scatter problem.

for each request pid:
  req_to_token[req_pool_indices[pid], seq_lens[pid] : seq_lens[pid]+L] =
      out_cache_loc[pid*L : (pid+1)*L]            (L = topk * spec_steps = 64)

Returns (new_req_to_token, out_cache_loc, empty, empty) like the reference.

Distribution: the 512 pool rows are block-sharded across 8 NeuronCores
(64 rows per core).  Each core copies its row-shard DRAM->DRAM and then
scatters the 64-element segments belonging to its rows via indirect
DMAs whose per-partition flat offsets (local_row * row_width + seq_len)
are computed on the host from the index inputs.  No cross-core traffic.

Raw Bass (no Tile).  The bulk copy is split over the two HWDGE dispatch
queues (SP + ACT, one big DMA each, ~5.25 MB per queue, 16 descriptors
-> all 16 SDMA engines); the staging load for the scatter records rides
the idle Pool SWDGE queue.  The indirect scatter waits on the copy
semaphores, runs on the Pool queue, and the gpsimd stream's final wait
makes it the last engine to halt, so no extra end-of-kernel rendezvous
is needed for correctness of the output.
"""

import os
from contextlib import ExitStack

import numpy as np

import concourse.bass as bass
import concourse.mybir as mybir
from concourse.bass import IndirectOffsetOnAxis
from concourse.bass_utils import run_bass_kernel_spmd

N_CORES = 8
NUM_POOLS = 512
POOL_LEN = 40960
B = 256
L = 64                      # topk * speculative_num_steps
ROWS_PER_CORE = NUM_POOLS // N_CORES    # 64
HALF_ROWS = ROWS_PER_CORE // 2          # 32
# scatter slots per half: each of the 32 rows in a half is hit by at most
# one request when req_pool_indices are distinct; 64 leaves 2x margin
PART = int(os.environ.get("KERNEL_PART", "64"))
OOB_PAD = 0x0FFFFFFF        # padding index, beyond bounds_check -> skipped

# --- tunables (bench.py mutates these; _get_program caches per config) ---
CHUNKS_PER_Q = int(os.environ.get("KERNEL_CHUNKS_PER_Q", "1"))
DESC_PER_CHUNK = int(os.environ.get("KERNEL_DESC_PER_CHUNK", "16"))
# rows copied by the Pool SWDGE queue (taken from the end); 0 = 2-queue copy
GPSIMD_ROWS = int(os.environ.get("KERNEL_GPSIMD_ROWS", "0"))
# scatter groups: 1 = one indirect DMA for all requests (waits both copy
# halves); 2 = one per row-half (first overlaps the second half's copy)
N_GROUPS = int(os.environ.get("KERNEL_N_GROUPS", "1"))
# rows on the SP queue; 0 = auto (half of what the HWDGE queues carry)
SYNC_ROWS = int(os.environ.get("KERNEL_SYNC_ROWS", "0"))
# block-interleave the two HWDGE rings so each SDMA engine's two streams
# cover adjacent address blocks instead of regions 5 MB apart
INTERLEAVE = os.environ.get("KERNEL_INTERLEAVE", "0") == "1"

_program_cache: dict[tuple, bass.Bass] = {}

# results of the most recent hardware run (BassKernelResults); test harness
# reads .exec_time_ns from here when tracing is enabled
LAST_RESULTS = None

# skip the all-engine barrier Bass.__init__ emits after its preamble: every
# cross-engine dependency in this kernel goes through explicit semaphores,
# so the copy/load DMAs can dispatch while the other engines are still
# booting instead of rendezvousing first
SKIP_INIT_BARRIER = os.environ.get("KERNEL_SKIP_INIT_BARRIER", "1") == "1"
# skip the all-engine barrier at Block exit too: the gpsimd stream is the
# last to halt (it waits for scatter completion, which waits for the
# copies), so execution-complete detection doesn't need a rendezvous
SKIP_EXIT_BARRIER = os.environ.get("KERNEL_SKIP_EXIT_BARRIER", "1") == "1"


class _LeanBass(bass.Bass):
    """Bass whose construction-time all_engine_barrier is a no-op."""

    _in_init = True
    _skip_barrier = False

    def __init__(self, *a, **k):
        super().__init__(*a, **k)
        self._in_init = False

    def all_engine_barrier(self, *a, **k):
        if self._in_init or self._skip_barrier:
            return
        return super().all_engine_barrier(*a, **k)


def _config():
    return (
        CHUNKS_PER_Q, DESC_PER_CHUNK, GPSIMD_ROWS, SKIP_INIT_BARRIER, PART,
        N_GROUPS, SKIP_EXIT_BARRIER, SYNC_ROWS, INTERLEAVE,
    )


def _build_program(lanes: int) -> bass.Bass:
    """Build the per-core Bass program.

    Everything on the device is int32; an int64 input is viewed as 2
    int32 lanes per element (lanes = itemsize // 4).
    """
    row_w = POOL_LEN * lanes            # int32 lanes per pool row
    total = ROWS_PER_CORE * row_w       # int32 lanes per core shard
    seg = L * lanes                     # int32 lanes per scatter segment

    # row ranges per copy path: sync, scalar, (optional) gpsimd
    g_rows = GPSIMD_ROWS
    hw_rows = ROWS_PER_CORE - g_rows
    sync_rows = SYNC_ROWS or (hw_rows + 1) // 2
    a, b = min(sync_rows, hw_rows), hw_rows  # sync [0,a) scalar [a,b) gps [b,64)
    if N_GROUPS == 2:
        assert a <= HALF_ROWS <= b

    nc = _LeanBass() if SKIP_INIT_BARRIER else bass.Bass()
    src = nc.dram_tensor(
        "src", [ROWS_PER_CORE, row_w], mybir.dt.int32, kind="ExternalInput"
    )
    # per-request scatter record: [offset, seg values...]; group g uses
    # rows [g*PART, (g+1)*PART)
    scat = nc.dram_tensor(
        "scat", [N_GROUPS * PART, 1 + seg], mybir.dt.int32, kind="ExternalInput"
    )
    dst = nc.dram_tensor(
        "dst", [ROWS_PER_CORE, row_w], mybir.dt.int32, kind="ExternalOutput"
    )

    src_flat = src[:, :].flatten()
    dst_flat = dst[:, :].flatten()

    def copy_chunks(eng, sem, r0, r1):
        """Emit CHUNKS_PER_Q dma_starts covering rows [r0, r1)."""
        if r0 >= r1:
            return
        lanes_lo = r0 * row_w
        span = (r1 - r0) * row_w
        n = CHUNKS_PER_Q
        for c in range(n):
            lo = lanes_lo + span * c // n
            hi = lanes_lo + span * (c + 1) // n
            d = DESC_PER_CHUNK
            while (hi - lo) % d:
                d //= 2
            eng.dma_start(
                out=dst_flat[lo:hi].rearrange("(d i) -> d i", d=d),
                in_=src_flat[lo:hi].rearrange("(d i) -> d i", d=d),
            ).then_inc(sem, 16)

    with ExitStack() as ctx:
        stage = [
            ctx.enter_context(
                nc.sbuf_tensor(f"stage{h}", [PART, 1 + seg], mybir.dt.int32)
            )
            for h in range(N_GROUPS)
        ]
        load_sem = ctx.enter_context(nc.semaphore("load_sem"))
        copy_sems = [
            ctx.enter_context(nc.semaphore(f"copy_sem{q}")) for q in range(3)
        ]
        sc_sem = ctx.enter_context(nc.semaphore("sc_sem"))
        block = ctx.enter_context(nc.Block(no_gpsimd_drain=True))

        interleave = INTERLEAVE and g_rows == 0 and CHUNKS_PER_Q == 1
        if interleave:
            assert N_GROUPS == 1
            blk = total // (2 * DESC_PER_CHUNK)
            s3 = src_flat.rearrange("(x p c) -> x p c", p=2, c=blk)
            d3 = dst_flat.rearrange("(x p c) -> x p c", p=2, c=blk)

        @block.sync
        def _(sync):
            if interleave:
                sync.dma_start(out=d3[:, 0, :], in_=s3[:, 0, :]).then_inc(
                    copy_sems[0], 16
                )
            else:
                copy_chunks(sync, copy_sems[0], 0, a)

        @block.scalar
        def _(scalar):
            if interleave:
                scalar.dma_start(out=d3[:, 1, :], in_=s3[:, 1, :]).then_inc(
                    copy_sems[1], 16
                )
            else:
                copy_chunks(scalar, copy_sems[1], a, b)

        @block.gpsimd
        def _(gpsimd):
            # stage the scatter records on the idle Pool queue so the
            # HWDGE queues dispatch their bulk copies immediately
            for h in range(N_GROUPS):
                gpsimd.dma_start(
                    out=stage[h][:, :], in_=scat[h * PART:(h + 1) * PART, :]
                ).then_inc(load_sem, 16)
            if g_rows:
                copy_chunks(gpsimd, copy_sems[2], b, ROWS_PER_CORE)
            gpsimd.wait_ge(load_sem, 16 * N_GROUPS)
            dst_ind = dst_flat.unsqueeze(1)  # [total, 1]

            def scatter(h):
                gpsimd.indirect_dma_start(
                    out=dst_ind,
                    out_offset=IndirectOffsetOnAxis(ap=stage[h][:, :1], axis=0),
                    in_=stage[h][:, 1:1 + seg],
                    in_offset=None,
                    bounds_check=total - seg,
                    oob_is_err=False,
                ).then_inc(sc_sem, 16)

            if N_GROUPS == 1:
                # one scatter covering all rows: wait for every copy queue
                if a > 0:
                    gpsimd.wait_ge(copy_sems[0], 16 * CHUNKS_PER_Q)
                if b > a:
                    gpsimd.wait_ge(copy_sems[1], 16 * CHUNKS_PER_Q)
                if g_rows:
                    gpsimd.wait_ge(copy_sems[2], 16 * CHUNKS_PER_Q)
                scatter(0)
            else:
                # half 0 = rows [0, 32): covered by sync (+scalar if a < 32)
                gpsimd.wait_ge(copy_sems[0], 16 * CHUNKS_PER_Q)
                if a < HALF_ROWS:
                    gpsimd.wait_ge(copy_sems[1], 16 * CHUNKS_PER_Q)
                scatter(0)
                # half 1 = rows [32, 64): covered by scalar (+gpsimd if b<64)
                gpsimd.wait_ge(copy_sems[1], 16 * CHUNKS_PER_Q)
                if g_rows:
                    gpsimd.wait_ge(copy_sems[2], 16 * CHUNKS_PER_Q)
                scatter(1)
            gpsimd.wait_ge(sc_sem, 16 * N_GROUPS)

        if SKIP_EXIT_BARRIER and SKIP_INIT_BARRIER:
            nc._skip_barrier = True
        try:
            ctx.close()
        finally:
            nc._skip_barrier = False

    return nc


def _get_program(lanes: int) -> bass.Bass:
    key = (lanes,) + _config()
    if key not in _program_cache:
        _program_cache[key] = _build_program(lanes)
    return _program_cache[key]


def kernel(
    req_pool_indices,
    req_to_token,
    seq_lens,
    extend_lens,
    num_new_pages_per_topk,
    out_cache_loc,
    last_page_lens_cumsum,
    duplicate_cache_len,
    topk,
    speculative_num_steps,
    page_size,
):
    global LAST_RESULTS

    rpt = np.ascontiguousarray(np.asarray(req_to_token))
    rpi = np.asarray(req_pool_indices)
    seq = np.asarray(seq_lens)
    ocl = np.ascontiguousarray(np.asarray(out_cache_loc))

    assert int(duplicate_cache_len) == 0
    assert int(page_size) == 1 or int(topk) == 1
    assert int(topk) * int(speculative_num_steps) == L
    assert rpt.shape == (NUM_POOLS, POOL_LEN)
    batch = seq.shape[0]
    assert batch <= B

    dtype = rpt.dtype
    itemsize = dtype.itemsize
    assert itemsize in (4, 8)
    lanes = itemsize // 4
    row_w = POOL_LEN * lanes
    seg = L * lanes

    rpt32 = rpt.view(np.int32).reshape(NUM_POOLS, row_w)
    vals32 = ocl.view(np.int32).reshape(batch, seg)

    rpi64 = rpi.astype(np.int64)
    assert rpi64.min() >= 0 and rpi64.max() < NUM_POOLS
    assert seq.min() >= 0 and seq.max() <= POOL_LEN - L
    core_of = rpi64 // ROWS_PER_CORE
    local_row = rpi64 % ROWS_PER_CORE
    half_of = local_row // HALF_ROWS
    start = local_row * row_w + seq.astype(np.int64) * lanes

    # grow the scatter capacity / group count if an unusual routing needs it
    global PART, N_GROUPS
    group_of = half_of if N_GROUPS == 2 else np.zeros_like(half_of)
    need = 1
    for c in range(N_CORES):
        for h in range(N_GROUPS):
            need = max(
                need, int(np.count_nonzero((core_of == c) & (group_of == h)))
            )
    if need > PART:
        if need <= 128:
            PART = 128
        else:
            assert N_GROUPS == 1 and need <= 256
            N_GROUPS, PART = 2, 128
            group_of = half_of
            for c in range(N_CORES):
                for h in range(N_GROUPS):
                    assert (
                        np.count_nonzero((core_of == c) & (group_of == h)) <= PART
                    ), "too many requests per row-half"

    in_maps = []
    for c in range(N_CORES):
        scat_arr = np.zeros((N_GROUPS * PART, 1 + seg), np.int32)
        scat_arr[:, 0] = OOB_PAD
        for h in range(N_GROUPS):
            sel = np.nonzero((core_of == c) & (group_of == h))[0]
            assert len(sel) <= PART
            scat_arr[h * PART: h * PART + len(sel), 0] = start[sel].astype(
                np.int32
            )
            scat_arr[h * PART: h * PART + len(sel), 1:] = vals32[sel]
        in_maps.append(
            {
                "src": np.ascontiguousarray(
                    rpt32[c * ROWS_PER_CORE:(c + 1) * ROWS_PER_CORE]
                ),
                "scat": scat_arr,
            }
        )

    nc = _get_program(lanes)
    trace = os.environ.get("KERNEL_TRACE", "0") == "1"
    LAST_RESULTS = run_bass_kernel_spmd(
        nc, in_maps, core_ids=list(range(N_CORES)), trace=trace
    )

    out32 = np.concatenate([r["dst"] for r in LAST_RESULTS.results])
    new_rpt = out32.view(dtype).reshape(NUM_POOLS, POOL_LEN)
    empty = np.zeros((0,), dtype=ocl.dtype)
    return (new_rpt, ocl, empty, empty)


# revision 43
# speedup vs baseline: 1.1485x; 1.1485x over previous
"""Trainium2 Bass kernel for the req_to_token scatter problem.

for each request pid:
  req_to_token[req_pool_indices[pid], seq_lens[pid] : seq_lens[pid]+L] =
      out_cache_loc[pid*L : (pid+1)*L]            (L = topk * spec_steps = 64)

Returns (new_req_to_token, out_cache_loc, empty, empty) like the reference.

Distribution: the 512 pool rows are block-sharded across 8 NeuronCores
(64 rows per core).  Each core copies its row-shard DRAM->DRAM and then
scatters the 64-element segments belonging to its rows via indirect
DMAs whose per-partition flat offsets (local_row * row_width + seq_len)
are computed on the host from the index inputs.  No cross-core traffic.

Raw Bass (no Tile).  The bulk copy is split over the two HWDGE dispatch
queues (SP + ACT, one big DMA each, ~5.25 MB per queue, 16 descriptors
-> all 16 SDMA engines); the staging load for the scatter records rides
the idle Pool SWDGE queue.  The indirect scatter waits on the copy
semaphores, runs on the Pool queue, and the gpsimd stream's final wait
makes it the last engine to halt, so no extra end-of-kernel rendezvous
is needed for correctness of the output.
"""

import os
from contextlib import ExitStack

import numpy as np

import concourse.bass as bass
import concourse.mybir as mybir
from concourse.bass import IndirectOffsetOnAxis
from concourse.bass_utils import run_bass_kernel_spmd

N_CORES = 8
NUM_POOLS = 512
POOL_LEN = 40960
B = 256
L = 64                      # topk * speculative_num_steps
ROWS_PER_CORE = NUM_POOLS // N_CORES    # 64
HALF_ROWS = ROWS_PER_CORE // 2          # 32
# scatter slots per half: each of the 32 rows in a half is hit by at most
# one request when req_pool_indices are distinct; 64 leaves 2x margin
PART = int(os.environ.get("KERNEL_PART", "64"))
OOB_PAD = 0x0FFFFFFF        # padding index, beyond bounds_check -> skipped

# --- tunables (bench.py mutates these; _get_program caches per config) ---
CHUNKS_PER_Q = int(os.environ.get("KERNEL_CHUNKS_PER_Q", "1"))
DESC_PER_CHUNK = int(os.environ.get("KERNEL_DESC_PER_CHUNK", "16"))
# rows copied by the Pool SWDGE queue (taken from the end); 0 = 2-queue copy
GPSIMD_ROWS = int(os.environ.get("KERNEL_GPSIMD_ROWS", "0"))
# scatter groups: 1 = one indirect DMA for all requests (waits both copy
# halves); 2 = one per row-half (first overlaps the second half's copy)
N_GROUPS = int(os.environ.get("KERNEL_N_GROUPS", "1"))
# rows on the SP queue; 0 = auto (half of what the HWDGE queues carry)
SYNC_ROWS = int(os.environ.get("KERNEL_SYNC_ROWS", "0"))
# block-interleave the two HWDGE rings so each SDMA engine's two streams
# cover adjacent address blocks instead of regions 5 MB apart
INTERLEAVE = os.environ.get("KERNEL_INTERLEAVE", "0") == "1"
# drop the unused PE/DVE engines' (preamble-only) instruction streams from
# the BIR so codegen emits no boot stub for them
STRIP_ENGINES = os.environ.get("KERNEL_STRIP_ENGINES", "0") == "1"

_program_cache: dict[tuple, bass.Bass] = {}

# results of the most recent hardware run (BassKernelResults); test harness
# reads .exec_time_ns from here when tracing is enabled
LAST_RESULTS = None

# skip the all-engine barrier Bass.__init__ emits after its preamble: every
# cross-engine dependency in this kernel goes through explicit semaphores,
# so the copy/load DMAs can dispatch while the other engines are still
# booting instead of rendezvousing first
SKIP_INIT_BARRIER = os.environ.get("KERNEL_SKIP_INIT_BARRIER", "1") == "1"
# skip the all-engine barrier at Block exit too: the gpsimd stream is the
# last to halt (it waits for scatter completion, which waits for the
# copies), so execution-complete detection doesn't need a rendezvous
SKIP_EXIT_BARRIER = os.environ.get("KERNEL_SKIP_EXIT_BARRIER", "1") == "1"


class _LeanBass(bass.Bass):
    """Bass whose construction-time all_engine_barrier is a no-op."""

    _in_init = True
    _skip_barrier = False

    def __init__(self, *a, **k):
        super().__init__(*a, **k)
        self._in_init = False

    def all_engine_barrier(self, *a, **k):
        if self._in_init or self._skip_barrier:
            return
        return super().all_engine_barrier(*a, **k)


def _config():
    return (
        CHUNKS_PER_Q, DESC_PER_CHUNK, GPSIMD_ROWS, SKIP_INIT_BARRIER, PART,
        N_GROUPS, SKIP_EXIT_BARRIER, SYNC_ROWS, INTERLEAVE,
    )


def _build_program(lanes: int) -> bass.Bass:
    """Build the per-core Bass program.

    Everything on the device is int32; an int64 input is viewed as 2
    int32 lanes per element (lanes = itemsize // 4).
    """
    row_w = POOL_LEN * lanes            # int32 lanes per pool row
    total = ROWS_PER_CORE * row_w       # int32 lanes per core shard
    seg = L * lanes                     # int32 lanes per scatter segment

    # row ranges per copy path: sync, scalar, (optional) gpsimd
    g_rows = GPSIMD_ROWS
    hw_rows = ROWS_PER_CORE - g_rows
    sync_rows = SYNC_ROWS or (hw_rows + 1) // 2
    a, b = min(sync_rows, hw_rows), hw_rows  # sync [0,a) scalar [a,b) gps [b,64)
    if N_GROUPS == 2:
        assert a <= HALF_ROWS <= b

    nc = _LeanBass() if SKIP_INIT_BARRIER else bass.Bass()
    src = nc.dram_tensor(
        "src", [ROWS_PER_CORE, row_w], mybir.dt.int32, kind="ExternalInput"
    )
    # per-request scatter record: [offset, seg values...]; group g uses
    # rows [g*PART, (g+1)*PART)
    scat = nc.dram_tensor(
        "scat", [N_GROUPS * PART, 1 + seg], mybir.dt.int32, kind="ExternalInput"
    )
    dst = nc.dram_tensor(
        "dst", [ROWS_PER_CORE, row_w], mybir.dt.int32, kind="ExternalOutput"
    )

    src_flat = src[:, :].flatten()
    dst_flat = dst[:, :].flatten()

    def copy_chunks(eng, sem, r0, r1):
        """Emit CHUNKS_PER_Q dma_starts covering rows [r0, r1)."""
        if r0 >= r1:
            return
        lanes_lo = r0 * row_w
        span = (r1 - r0) * row_w
        n = CHUNKS_PER_Q
        for c in range(n):
            lo = lanes_lo + span * c // n
            hi = lanes_lo + span * (c + 1) // n
            d = DESC_PER_CHUNK
            while (hi - lo) % d:
                d //= 2
            eng.dma_start(
                out=dst_flat[lo:hi].rearrange("(d i) -> d i", d=d),
                in_=src_flat[lo:hi].rearrange("(d i) -> d i", d=d),
            ).then_inc(sem, 16)

    with ExitStack() as ctx:
        stage = [
            ctx.enter_context(
                nc.sbuf_tensor(f"stage{h}", [PART, 1 + seg], mybir.dt.int32)
            )
            for h in range(N_GROUPS)
        ]
        load_sem = ctx.enter_context(nc.semaphore("load_sem"))
        copy_sems = [
            ctx.enter_context(nc.semaphore(f"copy_sem{q}")) for q in range(3)
        ]
        sc_sem = ctx.enter_context(nc.semaphore("sc_sem"))
        block = ctx.enter_context(nc.Block(no_gpsimd_drain=True))

        interleave = INTERLEAVE and g_rows == 0 and CHUNKS_PER_Q == 1
        if interleave:
            assert N_GROUPS == 1
            blk = total // (2 * DESC_PER_CHUNK)
            s3 = src_flat.rearrange("(x p c) -> x p c", p=2, c=blk)
            d3 = dst_flat.rearrange("(x p c) -> x p c", p=2, c=blk)

        @block.sync
        def _(sync):
            if interleave:
                sync.dma_start(out=d3[:, 0, :], in_=s3[:, 0, :]).then_inc(
                    copy_sems[0], 16
                )
            else:
                copy_chunks(sync, copy_sems[0], 0, a)

        @block.scalar
        def _(scalar):
            if interleave:
                scalar.dma_start(out=d3[:, 1, :], in_=s3[:, 1, :]).then_inc(
                    copy_sems[1], 16
                )
            else:
                copy_chunks(scalar, copy_sems[1], a, b)

        @block.gpsimd
        def _(gpsimd):
            # stage the scatter records on the idle Pool queue so the
            # HWDGE queues dispatch their bulk copies immediately
            for h in range(N_GROUPS):
                gpsimd.dma_start(
                    out=stage[h][:, :], in_=scat[h * PART:(h + 1) * PART, :]
                ).then_inc(load_sem, 16)
            if g_rows:
                copy_chunks(gpsimd, copy_sems[2], b, ROWS_PER_CORE)
            gpsimd.wait_ge(load_sem, 16 * N_GROUPS)
            dst_ind = dst_flat.unsqueeze(1)  # [total, 1]
            # materialize the bounds register during the copy window, not in
            # the post-copy-wait critical tail
            bounds_reg = gpsimd.to_reg(total - seg)

            def scatter(h):
                gpsimd.indirect_dma_start(
                    out=dst_ind,
                    out_offset=IndirectOffsetOnAxis(ap=stage[h][:, :1], axis=0),
                    in_=stage[h][:, 1:1 + seg],
                    in_offset=None,
                    bounds_check=bounds_reg,
                    oob_is_err=False,
                ).then_inc(sc_sem, 16)

            if N_GROUPS == 1:
                # one scatter covering all rows: wait for every copy queue
                if a > 0:
                    gpsimd.wait_ge(copy_sems[0], 16 * CHUNKS_PER_Q)
                if b > a:
                    gpsimd.wait_ge(copy_sems[1], 16 * CHUNKS_PER_Q)
                if g_rows:
                    gpsimd.wait_ge(copy_sems[2], 16 * CHUNKS_PER_Q)
                scatter(0)
            else:
                # half 0 = rows [0, 32): covered by sync (+scalar if a < 32)
                gpsimd.wait_ge(copy_sems[0], 16 * CHUNKS_PER_Q)
                if a < HALF_ROWS:
                    gpsimd.wait_ge(copy_sems[1], 16 * CHUNKS_PER_Q)
                scatter(0)
                # half 1 = rows [32, 64): covered by scalar (+gpsimd if b<64)
                gpsimd.wait_ge(copy_sems[1], 16 * CHUNKS_PER_Q)
                if g_rows:
                    gpsimd.wait_ge(copy_sems[2], 16 * CHUNKS_PER_Q)
                scatter(1)
            gpsimd.wait_ge(sc_sem, 16 * N_GROUPS)

        if SKIP_EXIT_BARRIER and SKIP_INIT_BARRIER:
            nc._skip_barrier = True
        try:
            ctx.close()
        finally:
            nc._skip_barrier = False

    if STRIP_ENGINES:
        dead = (mybir.EngineType.PE, mybir.EngineType.DVE)
        for f in nc.m.functions:
            for bb in f.blocks:
                bb.instructions = [
                    i for i in bb.instructions
                    if getattr(i, "engine", None) not in dead
                ]
    return nc


def _get_program(lanes: int) -> bass.Bass:
    key = (lanes,) + _config()
    if key not in _program_cache:
        _program_cache[key] = _build_program(lanes)
    return _program_cache[key]


def kernel(
    req_pool_indices,
    req_to_token,
    seq_lens,
    extend_lens,
    num_new_pages_per_topk,
    out_cache_loc,
    last_page_lens_cumsum,
    duplicate_cache_len,
    topk,
    speculative_num_steps,
    page_size,
):
    global LAST_RESULTS

    rpt = np.ascontiguousarray(np.asarray(req_to_token))
    rpi = np.asarray(req_pool_indices)
    seq = np.asarray(seq_lens)
    ocl = np.ascontiguousarray(np.asarray(out_cache_loc))

    assert int(duplicate_cache_len) == 0
    assert int(page_size) == 1 or int(topk) == 1
    assert int(topk) * int(speculative_num_steps) == L
    assert rpt.shape == (NUM_POOLS, POOL_LEN)
    batch = seq.shape[0]
    assert batch <= B

    dtype = rpt.dtype
    itemsize = dtype.itemsize
    assert itemsize in (4, 8)
    lanes = itemsize // 4
    row_w = POOL_LEN * lanes
    seg = L * lanes

    rpt32 = rpt.view(np.int32).reshape(NUM_POOLS, row_w)
    vals32 = ocl.view(np.int32).reshape(batch, seg)

    rpi64 = rpi.astype(np.int64)
    assert rpi64.min() >= 0 and rpi64.max() < NUM_POOLS
    assert seq.min() >= 0 and seq.max() <= POOL_LEN - L
    core_of = rpi64 // ROWS_PER_CORE
    local_row = rpi64 % ROWS_PER_CORE
    half_of = local_row // HALF_ROWS
    start = local_row * row_w + seq.astype(np.int64) * lanes

    # grow the scatter capacity / group count if an unusual routing needs it
    global PART, N_GROUPS
    group_of = half_of if N_GROUPS == 2 else np.zeros_like(half_of)
    need = 1
    for c in range(N_CORES):
        for h in range(N_GROUPS):
            need = max(
                need, int(np.count_nonzero((core_of == c) & (group_of == h)))
            )
    if need > PART:
        if need <= 128:
            PART = 128
        else:
            assert N_GROUPS == 1 and need <= 256
            N_GROUPS, PART = 2, 128
            group_of = half_of
            for c in range(N_CORES):
                for h in range(N_GROUPS):
                    assert (
                        np.count_nonzero((core_of == c) & (group_of == h)) <= PART
                    ), "too many requests per row-half"

    in_maps = []
    for c in range(N_CORES):
        scat_arr = np.zeros((N_GROUPS * PART, 1 + seg), np.int32)
        scat_arr[:, 0] = OOB_PAD
        for h in range(N_GROUPS):
            sel = np.nonzero((core_of == c) & (group_of == h))[0]
            assert len(sel) <= PART
            scat_arr[h * PART: h * PART + len(sel), 0] = start[sel].astype(
                np.int32
            )
            scat_arr[h * PART: h * PART + len(sel), 1:] = vals32[sel]
        in_maps.append(
            {
                "src": np.ascontiguousarray(
                    rpt32[c * ROWS_PER_CORE:(c + 1) * ROWS_PER_CORE]
                ),
                "scat": scat_arr,
            }
        )

    nc = _get_program(lanes)
    trace = os.environ.get("KERNEL_TRACE", "0") == "1"
    LAST_RESULTS = run_bass_kernel_spmd(
        nc, in_maps, core_ids=list(range(N_CORES)), trace=trace
    )

    out32 = np.concatenate([r["dst"] for r in LAST_RESULTS.results])
    new_rpt = out32.view(dtype).reshape(NUM_POOLS, POOL_LEN)
    empty = np.zeros((0,), dtype=ocl.dtype)
    return (new_rpt, ocl, empty, empty)
